# revision 57
# baseline (speedup 1.0000x reference)
"""Criss-cross (CCNet) sparse attention kernel for Trainium2, 8-core data-parallel.

Problem (hardcoded): B=8, CQ=64, CV=512, H=W=128, fp32 I/O.
Per core: one image.  reference:
    energy_H[i,w,j] = sum_c q[c,i,w] k[c,j,w]   (diag i==j masked -inf)
    energy_W[i,w,j] = sum_c q[c,i,w] k[c,i,j]
    att = softmax(concat(energy_H, energy_W), axis=j)  (256-way per pixel)
    out[c,i,w] = sum_j v[c,j,w] att_H[i,w,j] + sum_j v[c,i,j] att_W[i,w,j]

v3 strategy (minimize serialized DMA-device time; cost model charges out-AP
bytes with a 2x penalty under 512B runs and ~25us flat for big reordering
SWDGE casts — so all bulk DMA is kept layout-preserving):
  - q/k: gpsimd cast loads f32->fp16, natural layout, 1024-desc quarters.
  - v: gpsimd cast loads f32->bf16 in NATURAL [c, (i,j)] chunk layout
    (fully contiguous, 128 descriptors -> full bandwidth).
  - both PV operand layouts built on-chip by PE transposes of v_nat slices
    (bf16 stays bf16 through PSUM):
      vtW[j, (i, c)]  <- transpose(v_nat[:, i, :]) per row    (row-pass lhsT)
      natH[i, (w, c)] <- transpose(v_nat[:, :, w]) per column (col-pass lhsT)
    grouped 8 per PSUM bank-tile, then one packed bf16 copy -> SBUF
    (alternating DVE/ACT).
  - energies in [128, 1024] psum tiles; exp on ACT -> bf16 att maps
    att_W[j, (i,w)], att_H[j, (w,i)]; att_H's diagonal is killed by
    accumulating -30000*I onto the energy psum inside the matmul group
    (exp then underflows to exact 0 -- no DVE mask multiply);
    denominators via one-hot basis matmuls.
  - softmax scales via gpsimd partition_broadcast pieces + DVE bf16
    multiplies; att_H's reciprocal is produced 32 rows at a time DURING
    the col-energy half (per-block one-hot psum accumulators), so its
    scale wave finishes with phase 1a and the col PV pass starts
    immediately; att_W's wave follows on Pool.
  - PV col pass per w -> [c, i] psum -> copy (split ACT/DVE) ->
    colbuf[c, (w, i)]; row pass per i -> [c, w] psum; DVE add (row psum +
    strided colbuf) -> bf16 staging tile -> HWDGE store.
  - next-chunk transposes are interleaved INTO the col/row loops at group
    granularity (the tile framework's region-level WAR tracking lets each
    transpose group start as soon as its 8 rows of vtW/natH are consumed),
    keeping PE fed while DVE drains the merge adds.
  - output DRAM tensor is bf16 (halves store bytes); host upcasts to f32.
"""

import threading

import numpy as np

CQ, CV, H, W = 64, 512, 128, 128
PIX = H * W
B = 8
EXP_BIAS = -40.0
CHUNK = 128
N_CHUNKS = CV // CHUNK
G = 8              # slices per psum group tile
NG = 128 // G      # groups per chunk/map (16)
SC = 1024          # columns per broadcast/scale piece
NSC = PIX // SC    # pieces per map (16)


def build_nc():
    import concourse.mybir as mybir
    import concourse.tile as tile
    from concourse import bacc
    from concourse.masks import make_identity

    f32 = mybir.dt.float32
    bf16 = mybir.dt.bfloat16
    fp16 = mybir.dt.float16
    Exp = mybir.ActivationFunctionType.Exp
    add = mybir.AluOpType.add
    mult = mybir.AluOpType.mult

    nc = bacc.Bacc(None, target_bir_lowering=False)

    with tile.TileContext(nc) as tc:
        with (
            tc.tile_pool(name="dram", bufs=1, space="DRAM") as dram,
            tc.tile_pool(name="attp", bufs=1) as attp,
            tc.tile_pool(name="vnatp", bufs=1) as vnatp,
            tc.tile_pool(name="constp", bufs=1) as constp,
            tc.tile_pool(name="dnp", bufs=1) as dnp,
        ):
            q_d = dram.tile((CQ, H, W), f32, kind="ExternalInput", name="q", uniquify=False)
            k_d = dram.tile((CQ, H, W), f32, kind="ExternalInput", name="k", uniquify=False)
            v_d = dram.tile((CV, H, W), f32, kind="ExternalInput", name="v", uniquify=False)
            o_d = dram.tile((CV, H, W), bf16, kind="ExternalOutput", name="o", uniquify=False)

            # att_W[j, i*W + w] ; att_H[j, w*H + i]  (bf16, softmax-scaled)
            att_W = attp.tile([128, PIX], bf16)
            att_H = attp.tile([128, PIX], bf16)

            # v chunk in natural layout [c, (i, j)] bf16
            v_nat = vnatp.tile([128, H, W], bf16)

            # constants
            ident = constp.tile([128, 128], f32)
            make_identity(nc, ident[:])
            ident_bf = constp.tile([128, 128], bf16)
            nc.vector.tensor_copy(ident_bf[:], ident[:])
            # -30000*I: accumulated onto col-energy psum so exp() zeroes
            # the diagonal directly (no DVE mask multiply needed)
            negI = constp.tile([128, 128], bf16)
            nc.vector.tensor_scalar(negI[:], ident_bf[:], -30000.0, 0.0,
                                    op0=mult, op1=add)
            zb = constp.tile([128, 256], bf16)
            nc.vector.memset(zb[:], 0.0)
            nc.vector.memset(zb[:, 128:129], 1.0)
            bias_t = constp.tile([128, 1], f32)
            nc.vector.memset(bias_t[:], EXP_BIAS)

            # bf16 reciprocal maps (outlive phase 1a)
            rbf_iw = dnp.tile([128, 128], bf16, name="rbf_iw")
            rbf_wi = dnp.tile([128, 128], bf16, name="rbf_wi")

            def load_vnat(ck):
                # natural layout, contiguous: full-bandwidth cast DMA; small
                # pieces so tiny critical DMAs are not stuck in the FIFO
                for h in range(2):
                    nc.gpsimd.dma_start(
                        v_nat[h * 64:(h + 1) * 64].rearrange("c i j -> c (i j)"),
                        v_d[ck * CHUNK + h * 64:ck * CHUNK + (h + 1) * 64]
                        .rearrange("c i j -> c (i j)"),
                    )

            # ---- phase 1a: energies, exp, mask, denominators
            with (
                tc.tile_pool(name="qkp", bufs=1) as qkp,
                tc.tile_pool(name="rq1ap", bufs=2) as rq1ap,
                tc.tile_pool(name="prhp", bufs=2) as prhp,
                tc.tile_pool(name="pse", bufs=2, space="PSUM") as pse,
                tc.tile_pool(name="psdn", bufs=1, space="PSUM") as psdn,
            ):
                q_sb = qkp.tile([CQ, H, W], fp16)
                k_sb = qkp.tile([CQ, H, W], fp16)
                dnW_sb = qkp.tile([128, 128], f32, name="dnW_sb")
                dnH_sb = qkp.tile([128, 128], f32, name="dnH_sb")
                r_iw = qkp.tile([128, 128], f32, name="r_iw")
                r_wi = qkp.tile([128, 128], f32, name="r_wi")
                for r0 in range(0, H, 32):
                    nc.gpsimd.dma_start(q_sb[:, r0:r0 + 32, :], q_d[:, r0:r0 + 32, :])
                    nc.gpsimd.dma_start(k_sb[:, r0:r0 + 32, :], k_d[:, r0:r0 + 32, :])
                load_vnat(0)

                dnW_ps = psdn.tile([128, 128], f32, name="dnW_ps")

                for i0 in range(0, H, G):
                    pe = pse.tile([128, G * 128], f32, name="pe", tag="pe")
                    for d in range(G):
                        i = i0 + d
                        nc.tensor.matmul(
                            pe[:, d * 128:(d + 1) * 128],
                            lhsT=k_sb[:, i, :], rhs=q_sb[:, i, :],
                            start=True, stop=True,
                        )
                    nc.scalar.activation(
                        att_W[:, i0 * W:(i0 + G) * W], pe[:], Exp, bias=bias_t[:]
                    )
                    for d in range(G):
                        i = i0 + d
                        nc.tensor.matmul(
                            dnW_ps[:], lhsT=zb[:, 128 - i:256 - i],
                            rhs=att_W[:, i * W:(i + 1) * W],
                            start=(i == 0), stop=(i == H - 1),
                        )

                # dnW complete; transpose now so the col half can produce
                # r_wi (and scale att_H) group by group
                nc.vector.tensor_copy(dnW_sb[:], dnW_ps[:])
                t12 = psdn.tile([128, 256], f32, name="t12")
                t1 = t12[:, 0:128]
                nc.tensor.transpose(t1, dnW_sb[:], ident[:])  # [w, i]
                t1sb = qkp.tile([128, 128], f32, name="t1sb")
                nc.scalar.copy(t1sb[:], t1)

                BL = 32  # r/broadcast block: legal base partitions
                dng = None
                for w0 in range(0, W, G):
                    pe = pse.tile([128, G * 128], f32, name="pe", tag="pe")
                    for d in range(G):
                        w = w0 + d
                        nc.tensor.matmul(
                            pe[:, d * 128:(d + 1) * 128],
                            lhsT=k_sb[:, :, w], rhs=q_sb[:, :, w],
                            start=True, stop=False,
                        )
                        nc.tensor.matmul(
                            pe[:, d * 128:(d + 1) * 128],
                            lhsT=ident_bf[:], rhs=negI[:],
                            start=False, stop=True,
                        )
                    nc.scalar.activation(
                        att_H[:, w0 * H:(w0 + G) * H], pe[:], Exp, bias=bias_t[:]
                    )
                    sl = att_H[:, w0 * H:(w0 + G) * H]
                    # denominator rows accumulate into a per-32-block psum
                    # tile (one-hot basis -> rows independent)
                    if w0 % BL == 0:
                        dng = pse.tile([128, 128], f32, name="dng", tag="dng")
                    for d in range(G):
                        w = w0 + d
                        nc.tensor.matmul(
                            dng[:], lhsT=zb[:, 128 - w:256 - w],
                            rhs=att_H[:, w * H:(w + 1) * H],
                            start=(w % BL == 0), stop=(w % BL == BL - 1),
                        )
                    if w0 % BL == BL - G:
                        b = w0 - (BL - G)  # block start row
                        nc.vector.tensor_copy(dnH_sb[b:b + BL, :], dng[b:b + BL, :])
                        nc.vector.tensor_tensor(
                            r_wi[b:b + BL, :], dng[b:b + BL, :], t1sb[b:b + BL, :], op=add
                        )
                        nc.vector.reciprocal(r_wi[b:b + BL, :], r_wi[b:b + BL, :])
                        nc.vector.tensor_copy(rbf_wi[b:b + BL, :], r_wi[b:b + BL, :])
                        rq = rq1ap.tile([1, BL * 128], bf16, name="rq1a", tag="rq1a")
                        nc.sync.dma_start(rq[:], rbf_wi[b:b + BL, :])
                        for hh in range(2):
                            prh = prhp.tile([128, BL * 64], bf16, name="prh", tag="prh")
                            nc.gpsimd.partition_broadcast(
                                prh[:], rq[:, hh * BL * 64:(hh + 1) * BL * 64],
                                channels=128)
                            slb = att_H[:, b * H + hh * BL * 64:
                                        b * H + (hh + 1) * BL * 64]
                            nc.vector.tensor_tensor(slb, slb, prh[:], op=mult)

                # r_iw for att_W scaling (needs full dnH)
                t2 = t12[:, 128:256]
                nc.tensor.transpose(t2, dnH_sb[:], ident[:])  # [i, w]
                nc.vector.tensor_tensor(r_iw[:], t2, dnW_sb[:], op=add)
                nc.vector.reciprocal(r_iw[:], r_iw[:])
                nc.vector.tensor_copy(rbf_iw[:], r_iw[:])

            # ---- phase 1b + 2
            with (
                tc.tile_pool(name="r12qp", bufs=2) as r12qp,
                tc.tile_pool(name="prp", bufs=2) as prp,
                tc.tile_pool(name="vtWp", bufs=1) as vtWp,
                tc.tile_pool(name="natHp", bufs=1) as natHp,
                tc.tile_pool(name="colbufp", bufs=1) as colbufp,
                tc.tile_pool(name="stgp", bufs=2) as stgp,
                tc.tile_pool(name="pst", bufs=2, space="PSUM") as pst,
                tc.tile_pool(name="psc", bufs=2, space="PSUM") as psc,
                tc.tile_pool(name="psr", bufs=2, space="PSUM") as psr,
            ):
                # --- helpers -------------------------------------------------
                vtW = vtWp.tile([128, H, CHUNK], bf16)        # [j, i, c]
                natH = natHp.tile([128, W, CHUNK], bf16)      # [i, w, c]
                colbuf = colbufp.tile([128, W, H], bf16)      # [c, w, i]

                def trans_vtW(ck, share=4):
                    # vtW[j, (i8, c)] <- transpose(v_nat[:, i, :]) (row lhsT)
                    for g in range(NG):
                        pt = pst.tile([128, G * 128], bf16, name="pt", tag="pt")
                        for d in range(G):
                            i = g * G + d
                            nc.tensor.transpose(
                                pt[:, d * 128:(d + 1) * 128],
                                v_nat[:, i, :], ident_bf[:],
                            )
                        dst = vtW[:, g * G:(g + 1) * G, :].rearrange("j i c -> j (i c)")
                        if g % 8 < share:
                            nc.scalar.copy(dst, pt[:])
                        else:
                            nc.vector.tensor_copy(dst, pt[:])

                def trans_natH(ck, share=4):
                    # natH[i, (w8, c)] <- transpose(v_nat[:, :, w]) (col lhsT)
                    for g in range(NG):
                        pt = pst.tile([128, G * 128], bf16, name="pt", tag="pt")
                        for d in range(G):
                            w = g * G + d
                            nc.tensor.transpose(
                                pt[:, d * 128:(d + 1) * 128],
                                v_nat[:, :, w], ident_bf[:],
                            )
                        dst = natH[:, g * G:(g + 1) * G, :].rearrange("i w c -> i (w c)")
                        if g % 8 < share:
                            nc.scalar.copy(dst, pt[:])
                        else:
                            nc.vector.tensor_copy(dst, pt[:])

                # chunk-0 transposes overlap the att_W broadcast wave
                trans_natH(0)
                trans_vtW(0)
                # chunk-1 load: its Pool descriptor-gen must precede the
                # att_W broadcasts in Pool program order
                load_vnat(1)

                # --- att_W scaling: Pool broadcasts (row pass trails wave) --
                rows = SC // 128
                for e in range(NSC):
                    rq = r12qp.tile([1, SC], bf16, name="rq", tag="rq")
                    nc.sync.dma_start(rq[:], rbf_iw[e * rows:(e + 1) * rows, :])
                    pr = prp.tile([128, SC], bf16, name="pr", tag="pr")
                    nc.gpsimd.partition_broadcast(pr[:], rq[:], channels=128)
                    sl = att_W[:, e * SC:(e + 1) * SC]
                    nc.vector.tensor_tensor(sl, sl, pr[:], op=mult)

                # --- phase 2 chunk pipeline ---------------------------------
                GC = 4   # row psum group size (1-bank tiles)
                for ck in range(N_CHUNKS):
                    c0 = ck * CHUNK
                    # col pass: out_H[c, i] per w (copies split ACT/DVE so
                    # the time-to-last-copy that gates the adds halves)
                    for g in range(NG):
                        pc = psc.tile([128, G * 128], f32, name="pc", tag="pc")
                        for d in range(G):
                            w = g * G + d
                            nc.tensor.matmul(
                                pc[:, d * 128:(d + 1) * 128],
                                lhsT=natH[:, w, :],
                                rhs=att_H[:, w * H:(w + 1) * H],
                                start=True, stop=True,
                            )
                        dst = colbuf[:, g * G:(g + 1) * G, :].rearrange("c w i -> c (w i)")
                        if g % 2 == 0:
                            nc.scalar.copy(dst, pc[:])
                        else:
                            nc.vector.tensor_copy(dst, pc[:])
                        # next-chunk natH transpose group for the w-rows just
                        # consumed (region WAR: waits only this col group)
                        if ck + 1 < N_CHUNKS:
                            pt = pst.tile([128, G * 128], bf16, name="pt", tag="pt")
                            for d2 in range(G):
                                w2 = g * G + d2
                                nc.tensor.transpose(
                                    pt[:, d2 * 128:(d2 + 1) * 128],
                                    v_nat[:, :, w2], ident_bf[:],
                                )
                            dstn = natH[:, g * G:(g + 1) * G, :].rearrange("i w c -> i (w c)")
                            if g % 8 < 4:
                                nc.scalar.copy(dstn, pt[:])
                            else:
                                nc.vector.tensor_copy(dstn, pt[:])
                    # row pass + merge; store per pairs of groups
                    stg = None
                    for g in range(H // GC):
                        pr2 = psr.tile([128, GC * 128], f32, name="pr2", tag="pr2")
                        for d in range(GC):
                            i = g * GC + d
                            nc.tensor.matmul(
                                pr2[:, d * 128:(d + 1) * 128],
                                lhsT=vtW[:, i, :],
                                rhs=att_W[:, i * W:(i + 1) * W],
                                start=True, stop=True,
                            )
                        if g % 2 == 0:
                            stg = stgp.tile([128, 2 * GC * 128], bf16, name="stg", tag="stg")
                        half = (g % 2) * GC * 128
                        # stg[c, (i4, w)] = pr2 + colbuf[c, w, i4-range] (strided)
                        cb = colbuf[:, :, g * GC:(g + 1) * GC].transpose([0, 2, 1])
                        pv = pr2[:].rearrange("c (d w) -> c d w", d=GC)
                        sv = stg[:, half:half + GC * 128].rearrange("c (d w) -> c d w", d=GC)
                        nc.vector.tensor_tensor(sv, pv, cb, op=add)
                        if g % 2 == 1:
                            i0 = (g - 1) * GC
                            nc.sync.dma_start(
                                o_d[c0:c0 + CHUNK, i0:i0 + 2 * GC, :],
                                stg[:],
                            )
                            # next-chunk vtW transpose group for rows just
                            # consumed (region WAR: waits only those row mms)
                            if ck + 1 < N_CHUNKS:
                                tg = (g - 1) // 2
                                pt = pst.tile([128, G * 128], bf16, name="pt", tag="pt")
                                for d2 in range(G):
                                    i2 = tg * G + d2
                                    nc.tensor.transpose(
                                        pt[:, d2 * 128:(d2 + 1) * 128],
                                        v_nat[:, i2, :], ident_bf[:],
                                    )
                                dst2 = vtW[:, tg * G:(tg + 1) * G, :].rearrange("j i c -> j (i c)")
                                if tg % 2 == 0:
                                    nc.scalar.copy(dst2, pt[:])
                                else:
                                    nc.vector.tensor_copy(dst2, pt[:])
                    if ck + 2 < N_CHUNKS:
                        load_vnat(ck + 2)

    nc.compile()
    return nc


_CACHE = {}
_LOCK = threading.Lock()


def _get_nc():
    with _LOCK:
        if "nc" not in _CACHE:
            _CACHE["nc"] = build_nc()
        return _CACHE["nc"]


def kernel(proj_query: np.ndarray, proj_key: np.ndarray, proj_value: np.ndarray,
           trace: bool = False):
    from concourse.bass_utils import run_bass_kernel_spmd

    q = np.ascontiguousarray(np.asarray(proj_query, dtype=np.float32))
    k = np.ascontiguousarray(np.asarray(proj_key, dtype=np.float32))
    v = np.ascontiguousarray(np.asarray(proj_value, dtype=np.float32))
    assert q.shape == (B, CQ, H, W) and v.shape == (B, CV, H, W)

    nc = _get_nc()
    in_maps = [{"q": q[b], "k": k[b], "v": v[b]} for b in range(B)]
    res = run_bass_kernel_spmd(nc, in_maps, core_ids=list(range(B)), trace=trace)
    out = np.stack(
        [np.asarray(res.results[b]["o"]).astype(np.float32) for b in range(B)], axis=0
    )
    if trace:
        kernel.last_exec_time_ns = res.exec_time_ns
        kernel.last_results = res
    return out


if __name__ == "__main__":
    nc = build_nc()
    print("build ok:", nc)
